# revision 2
# baseline (speedup 1.0000x reference)
"""BitLinear (int8-style activation quant + ternary weight) for 8 TRN2 NeuronCores.

Data-parallel over tokens (1024 tokens/core). All quantization arithmetic is
done on the host (it is exactly the reference's math); the device runs a pure
bf16 GEMM with the ternary weight as the *stationary* PE operand and the
activations as the *moving* operand:

  out.T[o, t] = sum_k w_q[k, o] * (x_q[k, t] * c[t]),   c[t] = scale_w*scale_x[t]/QB

Folding the whole output scale c[t] into the bf16 activations makes the device
a scale-free GEMM (one bf16 rounding of x_q*c, ~1e-3 l2 vs the reference —
far inside the 2e-2 gate). The weight tile [128k, 128o] stays stationary for
two matmuls (the two 512-token halves), halving LDWEIGHTS pressure vs the
kc-innermost ordering. All tensors ship pre-permuted so every DMA descriptor
is >=2KB contiguous per partition. The [O, T] device output is untransposed on
the host.
"""

import os

import numpy as np

QB = 128.0
EPS = 1e-05

# A/B knob: drop InstLdweights that reload the identical stationary operand
# the PE already holds (the two 512-token halves share one weight tile).
DEDUP_LDW = os.environ.get("BITLIN_DEDUP", "1") == "1"
# Ternary weight as fp8e4 (exact for {-1,0,1}): halves the weight DMA stream
# and makes LDWEIGHTS 4x faster via FWL. HW-verified exact vs bf16 moving.
FP8_W = os.environ.get("BITLIN_FP8W", "1") == "1"
# Store the output in bf16 (halves store traffic; ~2^-9 extra rounding, far
# inside the 2e-2 gate); host upcasts to f32.
BF16_OUT = os.environ.get("BITLIN_BF16OUT", "1") == "1"

# Full-problem constants (hardcoded per harness contract).
N_CORES = 8
B, S, D_IN = 4, 2048, 4096
D_OUT = 4096
TOKENS = B * S                   # 8192
T = TOKENS // N_CORES            # 1024 tokens per core
P = 128
KC = D_IN // P                   # 32 contraction chunks
OB = D_OUT // P                  # 32 output-feature blocks
TH = T // 512                    # 2 moving-operand halves


def build_program(repeats=1, num_devices=N_CORES):
    """Per-core Bass program; all cores run this SPMD on their own shard."""
    import concourse.bacc as bacc
    import concourse.mybir as mybir
    import concourse.tile as tile

    f32 = mybir.dt.float32
    bf16 = mybir.dt.bfloat16
    wdt = mybir.dt.float8e4 if FP8_W else bf16
    odt = bf16 if BF16_OUT else f32

    nc = bacc.Bacc(
        "TRN2",
        target_bir_lowering=False,
        debug=False,
        enable_asserts=False,
        num_devices=num_devices,
    )
    # Layouts (partition dim first, >=2KB contiguous per partition per DMA):
    #   xs[p, kc, t]  = x_q[kc*128+p, t] * c[t]            (bf16)
    #   wq[p, ob, kc, o] = w_q[kc*128+p, ob*128+o]          (fp8e4/bf16)
    #   out[p, ob, t] = out_full[t, ob*128+p]               (bf16/f32)
    xs = nc.dram_tensor("xs", [P, KC, T], bf16, kind="ExternalInput").ap()
    wq = nc.dram_tensor("wq", [P, OB, KC, P], wdt, kind="ExternalInput").ap()
    out = nc.dram_tensor("out", [P, OB, T], odt, kind="ExternalOutput").ap()

    with tile.TileContext(nc) as tc:
        with (
            tc.tile_pool(name="xsp", bufs=2) as xsp,
            tc.tile_pool(name="wqp", bufs=3) as wqp,
            tc.tile_pool(name="outp", bufs=4) as outp,
            tc.tile_pool(name="psum", bufs=4, space="PSUM") as psump,
        ):
            def load_wq(ob, splits=1):
                # SWDGE ring: weight stream never queues behind x loads (SP
                # ring) or output stores (ACT ring).
                t_ = wqp.tile([P, KC, P], wdt, tag="wq")
                step = KC // splits
                for q in range(splits):
                    ks = slice(q * step, (q + 1) * step)
                    nc.gpsimd.dma_start(t_[:, ks, :], wq[:, ob, ks, :])
                return t_

            def body():
                xst = xsp.tile([P, KC, T], bf16, tag="xs")
                for q in range(4):
                    ks = slice(q * (KC // 4), (q + 1) * (KC // 4))
                    nc.sync.dma_start(xst[:, ks, :], xs[:, ks, :])
                wq_tiles = {0: load_wq(0, splits=4)}
                for ob in range(OB):
                    if ob + 1 < OB:
                        wq_tiles[ob + 1] = load_wq(ob + 1)
                    wt = wq_tiles.pop(ob)
                    ps = [
                        psump.tile([P, 512], f32, tag="ps", name=f"ps{th}")
                        for th in range(TH)
                    ]
                    for kc in range(KC):
                        for th in range(TH):
                            nc.tensor.matmul(
                                ps[th],
                                wt[:, kc, :],
                                xst[:, kc, th * 512 : (th + 1) * 512],
                                start=(kc == 0),
                                stop=(kc == KC - 1),
                            )
                    for th in range(TH):
                        ob_sb = outp.tile([P, 512], odt, tag="ob")
                        nc.scalar.activation(
                            ob_sb[:], ps[th][:], mybir.ActivationFunctionType.Copy
                        )
                        nc.scalar.dma_start(
                            out[:, ob, th * 512 : (th + 1) * 512], ob_sb[:]
                        )

            if repeats == 1:
                body()
            else:
                with tc.For_i(0, repeats, 1):
                    body()

    if DEDUP_LDW:
        _dedup_ldweights(nc, mybir)
    nc.compile()
    return nc


def _dedup_ldweights(nc, mybir):
    """Drop PE weight reloads whose stationary operand is already in the array.

    The PE stream is LDW,MM,LDW,MM,...; each weight tile is used by two
    consecutive matmuls, so every second LDW restreams identical data. A
    dropped LDW's waits are forwarded to the next kept PE instruction.
    """
    EngineType = type(nc.tensor.engine)
    removed = 0
    for blk in nc.main_func.blocks:
        keep = []
        last_key = None
        pending_waits = []
        for inst in blk.instructions:
            if getattr(inst, "engine", None) == EngineType.PE:
                if isinstance(inst, mybir.InstLdweights):
                    key = (
                        repr(inst.ins[0]),
                        getattr(inst, "perf_mode", None),
                        getattr(inst, "tile_position", None),
                    )
                    si = inst.sync_info
                    ups = [] if si is None else list(si.on_update or [])
                    if key == last_key and not ups:
                        if si is not None and si.on_wait:
                            pending_waits.extend(si.on_wait)
                        removed += 1
                        continue
                    last_key = key
                elif isinstance(inst, mybir.InstMatmult):
                    pass  # matmul does not clobber the loaded weights
                else:
                    last_key = None  # unknown PE instruction: be conservative
                if pending_waits:
                    si = inst.sync_info
                    if si is None:
                        inst.sync_info = mybir.SyncInfo(
                            on_wait=list(pending_waits), on_update=[]
                        )
                    else:
                        si.on_wait = list(si.on_wait or []) + pending_waits
                    pending_waits = []
            keep.append(inst)
        assert not pending_waits
        blk.instructions[:] = keep
    return removed


def host_prep(x, weight):
    """Quantization + scale folding + layout permutes, exactly as reference math."""
    import ml_dtypes

    xf = np.ascontiguousarray(x.reshape(TOKENS, D_IN), dtype=np.float32)
    w = np.asarray(weight, dtype=np.float32)

    # scale_w exactly as the jnp reference computes it (fp32 mean via XLA-CPU).
    try:
        import jax
        import jax.numpy as jnp

        cpu = jax.devices("cpu")[0]
        with jax.default_device(cpu):
            sw = np.float32(
                np.asarray(jnp.mean(jnp.abs(jax.device_put(w, cpu))) + EPS)
            )
    except Exception:
        sw = np.float32(np.mean(np.abs(w), dtype=np.float32) + np.float32(EPS))

    # Ternary weight, matching the reference's w_q (all ops fp32 IEEE).
    w_q = np.clip(np.round(w / sw), -1.0, 1.0).astype(np.float32)
    # wq[p, ob, kc, o] = w_q.T[kc*128+p, ob*128+o]
    wqT = np.ascontiguousarray(w_q.T)  # [K, N]
    w_dt = ml_dtypes.float8_e4m3 if FP8_W else ml_dtypes.bfloat16
    wq_dev = np.ascontiguousarray(
        wqT.reshape(KC, P, OB, P).transpose(1, 2, 0, 3)
    ).astype(w_dt)

    # Activation quantization (reference op order: (x*QB)/s, rne, clamp) and
    # output-scale folding: xs = bf16(x_q * c[t]), c = sw*s/QB.
    s = np.max(np.abs(xf), axis=1) + np.float32(EPS)            # [TOKENS] f32
    t_ = (xf * np.float32(QB)) / s[:, None]                      # f32, ref order
    x_q = np.clip(np.round(t_), -QB, QB)                         # ints (+-128 edge)
    c = (sw * s) / np.float32(QB)                                # [TOKENS] f32
    xs_all = (x_q * c[:, None]).astype(np.float32)

    in_maps = []
    for ci in range(N_CORES):
        lo, hi = ci * T, (ci + 1) * T
        # xs[p, kc, t] = xs_all[t, kc*128+p]
        xs_dev = np.ascontiguousarray(
            xs_all[lo:hi].reshape(T, KC, P).transpose(2, 1, 0)
        ).astype(ml_dtypes.bfloat16)
        in_maps.append({"xs": xs_dev, "wq": wq_dev})
    return in_maps


_nc_cache = {}


def _get_program(repeats=1):
    key = repeats
    if key not in _nc_cache:
        _nc_cache[key] = build_program(repeats=repeats)
    return _nc_cache[key]


def run_on_device(in_maps, repeats=1, retries=4):
    import time as _time

    from concourse.bass_utils import run_bass_kernel_spmd

    nc = _get_program(repeats)
    last = None
    for attempt in range(retries):
        try:
            return run_bass_kernel_spmd(
                nc, in_maps, core_ids=list(range(len(in_maps))), trace=False
            )
        except Exception as e:  # axon terminal occasionally drops a core; retry
            last = e
            _time.sleep(3 * (attempt + 1))
    raise last


def kernel(x, weight):
    in_maps = host_prep(x, weight)
    res = run_on_device(in_maps)
    full = np.empty((TOKENS, D_OUT), dtype=np.float32)
    for ci in range(N_CORES):
        m = np.asarray(res.results[ci]["out"], dtype=np.float32)
        # [P, OB, T] = out_full[t, ob*128+p]
        full[ci * T : (ci + 1) * T, :] = (
            m.transpose(1, 0, 2).reshape(D_OUT, T).T
        )
    return full.reshape(B, S, D_OUT)


# revision 3
# speedup vs baseline: 3.6968x; 3.6968x over previous
"""BitLinear (int8-style activation quant + ternary weight) for 8 TRN2 NeuronCores.

Data-parallel over tokens (1024 tokens/core). All quantization arithmetic is
done on the host (it is exactly the reference's math); the device runs a pure
GEMM — fp8e4 ternary weight as the *stationary* PE operand (exact for
{-1,0,1}; halves weight DMA, 4x faster FWL LDWEIGHTS) against bf16
activations as the *moving* operand (mixed-dtype matmul is HW-exact here):

  out.T[o, t] = sum_k w_q[k, o] * (x_q[k, t] * c[t]),   c[t] = scale_w*scale_x[t]/QB

Folding the whole output scale c[t] into the bf16 activations makes the device
a scale-free GEMM (one bf16 rounding of x_q*c, ~1e-3 l2 vs the reference —
far inside the 2e-2 gate). The weight tile [128k, 128o] stays stationary for
two matmuls (the two 512-token halves), halving LDWEIGHTS pressure vs the
kc-innermost ordering. All tensors ship pre-permuted so every DMA descriptor
is >=2KB contiguous per partition. The [O, T] device output is untransposed on
the host.
"""

import os

import numpy as np

QB = 128.0
EPS = 1e-05

# A/B knob: drop InstLdweights that reload the identical stationary operand
# the PE already holds (the two 512-token halves share one weight tile).
DEDUP_LDW = os.environ.get("BITLIN_DEDUP", "1") == "1"
# Ternary weight as fp8e4 (exact for {-1,0,1}): halves the weight DMA stream
# and makes LDWEIGHTS 4x faster via FWL. HW-verified exact vs bf16 moving.
FP8_W = os.environ.get("BITLIN_FP8W", "1") == "1"
# Store the output in bf16 (halves store traffic; ~2^-9 extra rounding, far
# inside the 2e-2 gate); host upcasts to f32.
BF16_OUT = os.environ.get("BITLIN_BF16OUT", "1") == "1"

# Full-problem constants (hardcoded per harness contract).
N_CORES = 8
B, S, D_IN = 4, 2048, 4096
D_OUT = 4096
TOKENS = B * S                   # 8192
T = TOKENS // N_CORES            # 1024 tokens per core
P = 128
KC = D_IN // P                   # 32 contraction chunks
OB = D_OUT // P                  # 32 output-feature blocks
TH = T // 512                    # 2 moving-operand halves


def build_program(repeats=1, num_devices=N_CORES):
    """Per-core Bass program; all cores run this SPMD on their own shard."""
    import concourse.bacc as bacc
    import concourse.mybir as mybir
    import concourse.tile as tile

    f32 = mybir.dt.float32
    bf16 = mybir.dt.bfloat16
    wdt = mybir.dt.float8e4 if FP8_W else bf16
    odt = bf16 if BF16_OUT else f32

    nc = bacc.Bacc(
        "TRN2",
        target_bir_lowering=False,
        debug=False,
        enable_asserts=False,
        num_devices=num_devices,
    )
    # Layouts (partition dim first, >=2KB contiguous per partition per DMA):
    #   xs[p, kc, t]  = x_q[kc*128+p, t] * c[t]            (bf16)
    #   wq[p, ob, kc, o] = w_q[kc*128+p, ob*128+o]          (fp8e4/bf16)
    #   out[p, ob, t] = out_full[t, ob*128+p]               (bf16/f32)
    xs = nc.dram_tensor("xs", [P, KC, T], bf16, kind="ExternalInput").ap()
    wq = nc.dram_tensor("wq", [P, OB, KC, P], wdt, kind="ExternalInput").ap()
    out = nc.dram_tensor("out", [P, OB, T], odt, kind="ExternalOutput").ap()

    with tile.TileContext(nc) as tc:
        with (
            tc.tile_pool(name="xsp", bufs=2) as xsp,
            tc.tile_pool(name="wqp", bufs=3) as wqp,
            tc.tile_pool(name="outp", bufs=4) as outp,
            tc.tile_pool(name="psum", bufs=4, space="PSUM") as psump,
        ):
            def load_wq(ob, splits=1):
                # SWDGE ring: weight stream never queues behind x loads (SP
                # ring) or output stores (ACT ring).
                t_ = wqp.tile([P, KC, P], wdt, tag="wq")
                step = KC // splits
                for q in range(splits):
                    ks = slice(q * step, (q + 1) * step)
                    nc.gpsimd.dma_start(t_[:, ks, :], wq[:, ob, ks, :])
                return t_

            def body():
                xst = xsp.tile([P, KC, T], bf16, tag="xs")
                for q in range(4):
                    ks = slice(q * (KC // 4), (q + 1) * (KC // 4))
                    nc.sync.dma_start(xst[:, ks, :], xs[:, ks, :])
                wq_tiles = {0: load_wq(0, splits=4)}
                for ob in range(OB):
                    if ob + 1 < OB:
                        wq_tiles[ob + 1] = load_wq(ob + 1)
                    wt = wq_tiles.pop(ob)
                    ps = [
                        psump.tile([P, 512], f32, tag="ps", name=f"ps{th}")
                        for th in range(TH)
                    ]
                    for kc in range(KC):
                        for th in range(TH):
                            nc.tensor.matmul(
                                ps[th],
                                wt[:, kc, :],
                                xst[:, kc, th * 512 : (th + 1) * 512],
                                start=(kc == 0),
                                stop=(kc == KC - 1),
                            )
                    for th in range(TH):
                        ob_sb = outp.tile([P, 512], odt, tag="ob")
                        nc.scalar.activation(
                            ob_sb[:], ps[th][:], mybir.ActivationFunctionType.Copy
                        )
                        nc.scalar.dma_start(
                            out[:, ob, th * 512 : (th + 1) * 512], ob_sb[:]
                        )

            if repeats == 1:
                body()
            else:
                with tc.For_i(0, repeats, 1):
                    body()

    if DEDUP_LDW:
        _dedup_ldweights(nc, mybir)
    nc.compile()
    return nc


def _dedup_ldweights(nc, mybir):
    """Drop PE weight reloads whose stationary operand is already in the array.

    The PE stream is LDW,MM,LDW,MM,...; each weight tile is used by two
    consecutive matmuls, so every second LDW restreams identical data. A
    dropped LDW's waits are forwarded to the next kept PE instruction.
    """
    EngineType = type(nc.tensor.engine)
    removed = 0
    for blk in nc.main_func.blocks:
        keep = []
        last_key = None
        pending_waits = []
        for inst in blk.instructions:
            if getattr(inst, "engine", None) == EngineType.PE:
                if isinstance(inst, mybir.InstLdweights):
                    key = (
                        repr(inst.ins[0]),
                        getattr(inst, "perf_mode", None),
                        getattr(inst, "tile_position", None),
                    )
                    si = inst.sync_info
                    ups = [] if si is None else list(si.on_update or [])
                    if key == last_key and not ups:
                        if si is not None and si.on_wait:
                            pending_waits.extend(si.on_wait)
                        removed += 1
                        continue
                    last_key = key
                elif isinstance(inst, mybir.InstMatmult):
                    pass  # matmul does not clobber the loaded weights
                else:
                    last_key = None  # unknown PE instruction: be conservative
                if pending_waits:
                    si = inst.sync_info
                    if si is None:
                        inst.sync_info = mybir.SyncInfo(
                            on_wait=list(pending_waits), on_update=[]
                        )
                    else:
                        si.on_wait = list(si.on_wait or []) + pending_waits
                    pending_waits = []
            keep.append(inst)
        assert not pending_waits
        blk.instructions[:] = keep
    return removed


def host_prep(x, weight):
    """Quantization + scale folding + layout permutes, exactly as reference math."""
    import ml_dtypes

    xf = np.ascontiguousarray(x.reshape(TOKENS, D_IN), dtype=np.float32)
    w = np.asarray(weight, dtype=np.float32)

    # scale_w exactly as the jnp reference computes it (fp32 mean via XLA-CPU).
    try:
        import jax
        import jax.numpy as jnp

        cpu = jax.devices("cpu")[0]
        with jax.default_device(cpu):
            sw = np.float32(
                np.asarray(jnp.mean(jnp.abs(jax.device_put(w, cpu))) + EPS)
            )
    except Exception:
        sw = np.float32(np.mean(np.abs(w), dtype=np.float32) + np.float32(EPS))

    # Ternary weight, matching the reference's w_q (all ops fp32 IEEE).
    w_q = np.clip(np.round(w / sw), -1.0, 1.0).astype(np.float32)
    # wq[p, ob, kc, o] = w_q.T[kc*128+p, ob*128+o]
    wqT = np.ascontiguousarray(w_q.T)  # [K, N]
    w_dt = ml_dtypes.float8_e4m3 if FP8_W else ml_dtypes.bfloat16
    wq_dev = np.ascontiguousarray(
        wqT.reshape(KC, P, OB, P).transpose(1, 2, 0, 3)
    ).astype(w_dt)

    # Activation quantization (reference op order: (x*QB)/s, rne, clamp) and
    # output-scale folding: xs = bf16(x_q * c[t]), c = sw*s/QB.
    s = np.max(np.abs(xf), axis=1) + np.float32(EPS)            # [TOKENS] f32
    t_ = (xf * np.float32(QB)) / s[:, None]                      # f32, ref order
    x_q = np.clip(np.round(t_), -QB, QB)                         # ints (+-128 edge)
    c = (sw * s) / np.float32(QB)                                # [TOKENS] f32
    xs_all = (x_q * c[:, None]).astype(np.float32)

    in_maps = []
    for ci in range(N_CORES):
        lo, hi = ci * T, (ci + 1) * T
        # xs[p, kc, t] = xs_all[t, kc*128+p]
        xs_dev = np.ascontiguousarray(
            xs_all[lo:hi].reshape(T, KC, P).transpose(2, 1, 0)
        ).astype(ml_dtypes.bfloat16)
        in_maps.append({"xs": xs_dev, "wq": wq_dev})
    return in_maps


_nc_cache = {}


def _get_program(repeats=1):
    key = repeats
    if key not in _nc_cache:
        _nc_cache[key] = build_program(repeats=repeats)
    return _nc_cache[key]


def run_on_device(in_maps, repeats=1, retries=4):
    import time as _time

    from concourse.bass_utils import run_bass_kernel_spmd

    nc = _get_program(repeats)
    last = None
    for attempt in range(retries):
        try:
            return run_bass_kernel_spmd(
                nc, in_maps, core_ids=list(range(len(in_maps))), trace=False
            )
        except Exception as e:  # axon terminal occasionally drops a core; retry
            last = e
            _time.sleep(3 * (attempt + 1))
    raise last


def kernel(x, weight):
    in_maps = host_prep(x, weight)
    res = run_on_device(in_maps)
    full = np.empty((TOKENS, D_OUT), dtype=np.float32)
    for ci in range(N_CORES):
        m = np.asarray(res.results[ci]["out"], dtype=np.float32)
        # [P, OB, T] = out_full[t, ob*128+p]
        full[ci * T : (ci + 1) * T, :] = (
            m.transpose(1, 0, 2).reshape(D_OUT, T).T
        )
    return full.reshape(B, S, D_OUT)
